# revision 4
# baseline (speedup 1.0000x reference)
"""Trainium2 Bass kernel for nn_GatedSpikingReservoirStep.

Reference computation (per batch row):
    prev = prev_state[:, :2048]
    input_part = inputs @ W_in.T                    # [B, R]
    reservoir_part = prev @ W_res.T                 # [B, R]
    gate = sigmoid(inputs @ W_gate.T)               # [B, 3R] -> i, f, o
    state = 0.9 * f * prev + 0.1 * tanh(i * (input_part + reservoir_part))
    state = o * state
    state = where(state > 0.5, state - 0.5, state)
    out = pad(state, [B, 2560])

Strategy: data-parallel over batch (8 cores x 512 rows). All matmuls are
computed transposed (out[r, b] = W_slice @ x_shard.T) so the contraction
dim (d or r') is the SBUF partition dim for both operands. The host
pre-packs every operand so each device DMA is per-partition contiguous.
Matmuls run in float32r (TF32-like multiply, fp32 accumulate, 4x the
fp32 rate). The gate/tanh/spike epilogue runs on ACT + DVE overlapped
with the next reservoir tile's matmuls.
"""

import numpy as np

B = 4096
D = 1024
R = 2048
MAX_DIM = 2560
N_CORES = 8
BS = B // N_CORES          # 512 batch rows per core
RT = R // 128              # 16 reservoir tiles of 128
KD = D // 128              # 8 contraction chunks over input dim
KR = R // 128              # 16 contraction chunks over reservoir dim

LEAK = 0.1
THRESH = 0.5

# 'f32r' (fast, ~1.5e-4 matmul rel err) or 'f32' (exact, 4x slower)
MM_MODE = 'f32r'

_cache = {}


def _build_nc():
    """Build and compile the per-core Bass module (same NEFF on all cores)."""
    import concourse.mybir as mybir
    import concourse.tile as tile
    from concourse import bacc

    F32 = mybir.dt.float32
    MMDT = mybir.dt.float32r if MM_MODE == 'f32r' else mybir.dt.float32
    AF = mybir.ActivationFunctionType
    OP = mybir.AluOpType

    nc = bacc.Bacc("TRN2", target_bir_lowering=False, debug=False)

    # Host-packed inputs; all are [128-partition, contiguous-free] blocks.
    x_d = nc.dram_tensor("x", [KD, 128, BS], MMDT, kind="ExternalInput")
    p_d = nc.dram_tensor("p", [KR, 128, BS], MMDT, kind="ExternalInput")
    win_d = nc.dram_tensor("win", [RT, 128, KD, 128], MMDT, kind="ExternalInput")
    wres_d = nc.dram_tensor("wres", [RT, 128, KR, 128], MMDT, kind="ExternalInput")
    wg_d = nc.dram_tensor("wg", [3, RT, 128, KD, 128], MMDT, kind="ExternalInput")
    out_d = nc.dram_tensor("out", [R, BS], F32, kind="ExternalOutput")

    with tile.TileContext(nc) as tc:
        with (
            tc.tile_pool(name="acts", bufs=1) as acts,
            tc.tile_pool(name="wpool", bufs=3) as wpool,
            tc.tile_pool(name="epi", bufs=2) as epi,
            tc.tile_pool(name="psum", bufs=2, space="PSUM") as psum,
        ):
            # --- tile-0 weight + input DMA front, most-urgent first.
            # sync queue: i-gate weights, then x chunks, rest of tile-0 weights.
            # scalar queue: prev chunks (needed only from the W_res group on).
            wg_ts = []
            win_ts = []
            wres_ts = []

            def load_wg(t):
                w = wpool.tile([128, 3, KD, 128], MMDT, tag="wg")
                for g in range(3):
                    nc.sync.dma_start(w[:, g], wg_d.ap()[g, t])
                wg_ts.append(w)

            def load_wg_g(t, g, w=None):
                if w is None:
                    w = wpool.tile([128, 3, KD, 128], MMDT, tag="wg")
                    wg_ts.append(w)
                nc.sync.dma_start(w[:, g], wg_d.ap()[g, t])
                return w

            def load_win(t):
                w = wpool.tile([128, KD, 128], MMDT, tag="win")
                nc.sync.dma_start(w[:], win_d.ap()[t])
                win_ts.append(w)

            def load_wres(t):
                w = wpool.tile([128, KR, 128], MMDT, tag="wres")
                nc.sync.dma_start(w[:], wres_d.ap()[t])
                wres_ts.append(w)

            # i-gate weights for tile 0 first: the very first real matmul
            # needs only this + x chunk 0.
            w0 = load_wg_g(0, 0)
            x_ks = []
            for k in range(KD):
                xk = acts.tile([128, BS], MMDT, tag=f"x{k}")
                nc.sync.dma_start(xk[:], x_d.ap()[k])
                x_ks.append(xk)
            load_wg_g(0, 1, w0)
            load_wg_g(0, 2, w0)
            load_win(0)
            load_wres(0)
            p_ks = []
            for k in range(KR):
                pk = acts.tile([128, BS], MMDT, tag=f"p{k}")
                nc.scalar.dma_start(pk[:], p_d.ap()[k])
                p_ks.append(pk)

            for t in range(RT):
                if t + 1 < RT:
                    load_wg(t + 1)
                    load_win(t + 1)
                    load_wres(t + 1)
                wg_t, win_t, wres_t = wg_ts[t], win_ts[t], wres_ts[t]

                ps_i = psum.tile([128, BS], F32, tag="ps_i")
                ps_f = psum.tile([128, BS], F32, tag="ps_f")
                ps_o = psum.tile([128, BS], F32, tag="ps_o")
                ps_s = psum.tile([128, BS], F32, tag="ps_s")

                # gate logits first (their weights arrive first); the
                # s = input_part + reservoir_part sum accumulates last so
                # prev chunks get the longest runway on tile 0.
                for k in range(KD):
                    nc.tensor.matmul(ps_i[:], wg_t[:, 0, k], x_ks[k][:],
                                     start=(k == 0), stop=(k == KD - 1))
                for k in range(KD):
                    nc.tensor.matmul(ps_f[:], wg_t[:, 1, k], x_ks[k][:],
                                     start=(k == 0), stop=(k == KD - 1))
                for k in range(KD):
                    nc.tensor.matmul(ps_o[:], wg_t[:, 2, k], x_ks[k][:],
                                     start=(k == 0), stop=(k == KD - 1))
                for k in range(KD):
                    nc.tensor.matmul(ps_s[:], win_t[:, k], x_ks[k][:],
                                     start=(k == 0), stop=False)
                for k in range(KR):
                    nc.tensor.matmul(ps_s[:], wres_t[:, k], p_ks[k][:],
                                     start=False, stop=(k == KR - 1))

                # epilogue: state = o*(0.9*f*prev + 0.1*tanh(i*s)), spike
                prev_t = p_ks[t][:]
                if MM_MODE == 'f32r':
                    prev_t = prev_t.bitcast(F32)
                si = epi.tile([128, BS], F32, tag="si")
                nc.scalar.activation(si[:], ps_i[:], AF.Sigmoid)
                sf = epi.tile([128, BS], F32, tag="sf")
                nc.scalar.activation(sf[:], ps_f[:], AF.Sigmoid)
                so = epi.tile([128, BS], F32, tag="so")
                nc.scalar.activation(so[:], ps_o[:], AF.Sigmoid)
                fp9 = epi.tile([128, BS], F32, tag="fp9")
                nc.vector.scalar_tensor_tensor(fp9[:], sf[:], 1.0 - LEAK, prev_t,
                                               OP.mult, OP.mult)
                x1 = epi.tile([128, BS], F32, tag="x1")
                nc.vector.tensor_tensor(x1[:], si[:], ps_s[:], OP.mult)
                th = epi.tile([128, BS], F32, tag="th")
                nc.scalar.activation(th[:], x1[:], AF.Tanh)
                pre = epi.tile([128, BS], F32, tag="pre")
                nc.vector.scalar_tensor_tensor(pre[:], th[:], LEAK, fp9[:],
                                               OP.mult, OP.add)
                st = epi.tile([128, BS], F32, tag="st")
                nc.vector.tensor_tensor(st[:], pre[:], so[:], OP.mult)
                msk = epi.tile([128, BS], F32, tag="msk")
                nc.vector.tensor_scalar(msk[:], st[:], THRESH, THRESH,
                                        OP.is_gt, OP.mult)
                ot = epi.tile([128, BS], F32, tag="ot")
                nc.vector.tensor_tensor(ot[:], st[:], msk[:], OP.subtract)
                nc.scalar.dma_start(out_d.ap()[t * 128:(t + 1) * 128], ot[:])

    nc.compile()
    return nc


def _get_nc():
    if 'nc' not in _cache:
        _cache['nc'] = _build_nc()
    return _cache['nc']


def _pack_inputs(inputs, prev_state, W_in, W_res, W_gate):
    """Host-side packing: transpose so contraction dim lands on SBUF
    partitions, with per-partition-contiguous DMA blocks."""
    f = np.float32
    # x[c, k, p, b] = inputs[512c + b, 128k + p]
    xp = np.ascontiguousarray(
        inputs.reshape(N_CORES, BS, KD, 128).transpose(0, 2, 3, 1).astype(f, copy=False))
    # p[c, k, p, b] = prev_state[512c + b, 128k + p]
    pp = np.ascontiguousarray(
        prev_state[:, :R].reshape(N_CORES, BS, KR, 128).transpose(0, 2, 3, 1).astype(f, copy=False))
    # win[t, p, k, m] = W_in[128t + m, 128k + p]
    win = np.ascontiguousarray(
        W_in.reshape(RT, 128, KD, 128).transpose(0, 3, 2, 1).astype(f, copy=False))
    # wres[t, p, j, m] = W_res[128t + m, 128j + p]
    wres = np.ascontiguousarray(
        W_res.reshape(RT, 128, KR, 128).transpose(0, 3, 2, 1).astype(f, copy=False))
    # wg[g, t, p, k, m] = W_gate[2048g + 128t + m, 128k + p]
    wg = np.ascontiguousarray(
        W_gate.reshape(3, RT, 128, KD, 128).transpose(0, 1, 4, 3, 2).astype(f, copy=False))

    in_maps = []
    for c in range(N_CORES):
        in_maps.append({
            "x": xp[c], "p": pp[c],
            "win": win, "wres": wres, "wg": wg,
        })
    return in_maps


def _assemble(results):
    out = np.zeros((B, MAX_DIM), dtype=np.float32)
    for c in range(N_CORES):
        out[c * BS:(c + 1) * BS, :R] = results[c]["out"].T
    return out


def _run(in_maps, **spmd_kwargs):
    from concourse.bass_utils import run_bass_kernel_spmd
    nc = _get_nc()
    return run_bass_kernel_spmd(nc, in_maps, core_ids=list(range(N_CORES)),
                                **spmd_kwargs)


def kernel(inputs, prev_state, W_in, W_res, W_gate):
    in_maps = _pack_inputs(inputs, prev_state, W_in, W_res, W_gate)
    res = _run(in_maps)
    return _assemble(res.results)


# revision 6
# speedup vs baseline: 1.0503x; 1.0503x over previous
"""Trainium2 Bass kernel for nn_GatedSpikingReservoirStep.

Reference computation (per batch row):
    prev = prev_state[:, :2048]
    input_part = inputs @ W_in.T                    # [B, R]
    reservoir_part = prev @ W_res.T                 # [B, R]
    gate = sigmoid(inputs @ W_gate.T)               # [B, 3R] -> i, f, o
    state = 0.9 * f * prev + 0.1 * tanh(i * (input_part + reservoir_part))
    state = o * state
    state = where(state > 0.5, state - 0.5, state)
    out = pad(state, [B, 2560])

Strategy: data-parallel over batch (8 cores x 512 rows). All matmuls are
computed transposed (out[r, b] = W_slice @ x_shard.T) so the contraction
dim (d or r') is the SBUF partition dim for both operands. The host
pre-packs every operand so each device DMA is per-partition contiguous.
Matmuls run in float32r (TF32-like multiply, fp32 accumulate, 4x the
fp32 rate). Software pipeline: the three gate GEMMs for reservoir tile
t+1 run ahead of the state GEMM for tile t, so the early tiles only
wait on x + gate weights while prev/W_in/W_res stream in; the
gate/tanh/spike epilogue runs on ACT + DVE one tile behind the PE.
"""

import numpy as np

B = 4096
D = 1024
R = 2048
MAX_DIM = 2560
N_CORES = 8
BS = B // N_CORES          # 512 batch rows per core
RT = R // 128              # 16 reservoir tiles of 128
KD = D // 128              # 8 contraction chunks over input dim
KR = R // 128              # 16 contraction chunks over reservoir dim

LEAK = 0.1
THRESH = 0.5

# 'f32r' (fast, ~1.5e-4 matmul rel err) or 'f32' (exact, 4x slower)
MM_MODE = 'f32r'

_cache = {}


def _build_nc():
    """Build and compile the per-core Bass module (same NEFF on all cores)."""
    import concourse.mybir as mybir
    import concourse.tile as tile
    from concourse import bacc

    F32 = mybir.dt.float32
    MMDT = mybir.dt.float32r if MM_MODE == 'f32r' else mybir.dt.float32
    AF = mybir.ActivationFunctionType
    OP = mybir.AluOpType

    nc = bacc.Bacc("TRN2", target_bir_lowering=False, debug=False)

    # Host-packed inputs; all are [128-partition, contiguous-free] blocks.
    x_d = nc.dram_tensor("x", [KD, 128, BS], MMDT, kind="ExternalInput")
    p_d = nc.dram_tensor("p", [KR, 128, BS], MMDT, kind="ExternalInput")
    win_d = nc.dram_tensor("win", [RT, 128, KD, 128], MMDT, kind="ExternalInput")
    wres_d = nc.dram_tensor("wres", [RT, 128, KR, 128], MMDT, kind="ExternalInput")
    wg_d = nc.dram_tensor("wg", [3, RT, 128, KD, 128], MMDT, kind="ExternalInput")
    out_d = nc.dram_tensor("out", [R, BS], F32, kind="ExternalOutput")

    with tile.TileContext(nc) as tc:
        with (
            tc.tile_pool(name="acts", bufs=1) as acts,
            tc.tile_pool(name="wpool", bufs=3) as wpool,
            tc.tile_pool(name="epi", bufs=2) as epi,
            tc.tile_pool(name="psum", bufs=2, space="PSUM") as psum,
        ):
            wg_ts = {}
            win_ts = {}
            wres_ts = {}
            x_ks = []
            p_ks = []

            def load_wg_g(t, g):
                w = wg_ts.get(t)
                if w is None:
                    w = wpool.tile([128, 3, KD, 128], MMDT, tag="wg",
                                   name=f"wg{t}")
                    wg_ts[t] = w
                nc.sync.dma_start(w[:, g], wg_d.ap()[g, t])

            def load_win(t):
                w = wpool.tile([128, KD, 128], MMDT, tag="win", name=f"win{t}")
                win_ts[t] = w
                nc.sync.dma_start(w[:], win_d.ap()[t])

            def load_wres(t):
                w = wpool.tile([128, KR, 128], MMDT, tag="wres", name=f"wres{t}")
                wres_ts[t] = w
                nc.sync.dma_start(w[:], wres_d.ap()[t])

            # ---- DMA front, single queue, in order of first PE use.
            load_wg_g(0, 0)
            for k in range(KD):
                xk = acts.tile([128, BS], MMDT, tag=f"x{k}")
                nc.sync.dma_start(xk[:], x_d.ap()[k])
                x_ks.append(xk)
            load_wg_g(0, 1)
            load_wg_g(0, 2)
            load_wg_g(1, 0)
            load_wg_g(1, 1)
            load_wg_g(1, 2)
            load_win(0)
            load_wres(0)
            for k in range(KR):
                pk = acts.tile([128, BS], MMDT, tag=f"p{k}")
                nc.sync.dma_start(pk[:], p_d.ap()[k])
                p_ks.append(pk)

            ps_gates = {}

            def gate_mms(t):
                wg_t = wg_ts[t]
                ps_i = psum.tile([128, BS], F32, tag="ps_i")
                ps_f = psum.tile([128, BS], F32, tag="ps_f")
                ps_o = psum.tile([128, BS], F32, tag="ps_o")
                ps_gates[t] = (ps_i, ps_f, ps_o)
                for g, ps in enumerate((ps_i, ps_f, ps_o)):
                    for k in range(KD):
                        nc.tensor.matmul(ps[:], wg_t[:, g, k], x_ks[k][:],
                                         start=(k == 0), stop=(k == KD - 1))

            def epilogue(t, ps_s, lo, n):
                """state = o*(0.9*f*prev + 0.1*tanh(i*s)) + spike, columns
                [lo, lo+n)."""
                ps_i, ps_f, ps_o = ps_gates[t]
                sl = np.s_[:, lo:lo + n]
                prev_t = p_ks[t][sl]
                if MM_MODE == 'f32r':
                    prev_t = prev_t.bitcast(F32)
                si = epi.tile([128, BS], F32, tag="si")
                nc.scalar.activation(si[sl], ps_i[sl], AF.Sigmoid)
                sf = epi.tile([128, BS], F32, tag="sf")
                nc.scalar.activation(sf[sl], ps_f[sl], AF.Sigmoid)
                so = epi.tile([128, BS], F32, tag="so")
                nc.scalar.activation(so[sl], ps_o[sl], AF.Sigmoid)
                fp9 = epi.tile([128, BS], F32, tag="fp9")
                nc.vector.scalar_tensor_tensor(fp9[sl], sf[sl], 1.0 - LEAK,
                                               prev_t, OP.mult, OP.mult)
                x1 = epi.tile([128, BS], F32, tag="x1")
                nc.vector.tensor_tensor(x1[sl], si[sl], ps_s[sl], OP.mult)
                th = epi.tile([128, BS], F32, tag="th")
                nc.scalar.activation(th[sl], x1[sl], AF.Tanh)
                pre = epi.tile([128, BS], F32, tag="pre")
                nc.vector.scalar_tensor_tensor(pre[sl], th[sl], LEAK, fp9[sl],
                                               OP.mult, OP.add)
                st = epi.tile([128, BS], F32, tag="st")
                nc.vector.tensor_tensor(st[sl], pre[sl], so[sl], OP.mult)
                msk = epi.tile([128, BS], F32, tag="msk")
                nc.vector.tensor_scalar(msk[sl], st[sl], THRESH, THRESH,
                                        OP.is_gt, OP.mult)
                ot = epi.tile([128, BS], F32, tag="ot")
                nc.vector.tensor_tensor(ot[sl], st[sl], msk[sl], OP.subtract)
                nc.scalar.dma_start(out_d.ap()[t * 128:(t + 1) * 128, lo:lo + n],
                                    ot[sl])

            # ---- pipelined main loop: gates for t+1, then state GEMM +
            # epilogue for t.
            gate_mms(0)
            for t in range(RT):
                # prefetch loads, in next-use order
                if t + 2 < RT:
                    for g in range(3):
                        load_wg_g(t + 2, g)
                if t + 1 < RT:
                    load_win(t + 1)
                    load_wres(t + 1)

                if t + 1 < RT:
                    gate_mms(t + 1)

                # s = input_part + reservoir_part for tile t
                win_t, wres_t = win_ts.pop(t), wres_ts.pop(t)
                ps_s = psum.tile([128, BS], F32, tag="ps_s")
                for k in range(KD):
                    nc.tensor.matmul(ps_s[:], win_t[:, k], x_ks[k][:],
                                     start=(k == 0), stop=False)
                for k in range(KR):
                    nc.tensor.matmul(ps_s[:], wres_t[:, k], p_ks[k][:],
                                     start=False, stop=(k == KR - 1))

                if t == RT - 1:
                    # last tile: halve the epilogue so its serial chain
                    # off the final matmul is shorter
                    epilogue(t, ps_s, 0, BS // 2)
                    epilogue(t, ps_s, BS // 2, BS // 2)
                else:
                    epilogue(t, ps_s, 0, BS)
                del ps_gates[t]

    nc.compile()
    return nc


def _get_nc():
    if 'nc' not in _cache:
        _cache['nc'] = _build_nc()
    return _cache['nc']


def _pack_inputs(inputs, prev_state, W_in, W_res, W_gate):
    """Host-side packing: transpose so contraction dim lands on SBUF
    partitions, with per-partition-contiguous DMA blocks."""
    f = np.float32
    # x[c, k, p, b] = inputs[512c + b, 128k + p]
    xp = np.ascontiguousarray(
        inputs.reshape(N_CORES, BS, KD, 128).transpose(0, 2, 3, 1).astype(f, copy=False))
    # p[c, k, p, b] = prev_state[512c + b, 128k + p]
    pp = np.ascontiguousarray(
        prev_state[:, :R].reshape(N_CORES, BS, KR, 128).transpose(0, 2, 3, 1).astype(f, copy=False))
    # win[t, p, k, m] = W_in[128t + m, 128k + p]
    win = np.ascontiguousarray(
        W_in.reshape(RT, 128, KD, 128).transpose(0, 3, 2, 1).astype(f, copy=False))
    # wres[t, p, j, m] = W_res[128t + m, 128j + p]
    wres = np.ascontiguousarray(
        W_res.reshape(RT, 128, KR, 128).transpose(0, 3, 2, 1).astype(f, copy=False))
    # wg[g, t, p, k, m] = W_gate[2048g + 128t + m, 128k + p]
    wg = np.ascontiguousarray(
        W_gate.reshape(3, RT, 128, KD, 128).transpose(0, 1, 4, 3, 2).astype(f, copy=False))

    in_maps = []
    for c in range(N_CORES):
        in_maps.append({
            "x": xp[c], "p": pp[c],
            "win": win, "wres": wres, "wg": wg,
        })
    return in_maps


def _assemble(results):
    out = np.zeros((B, MAX_DIM), dtype=np.float32)
    for c in range(N_CORES):
        out[c * BS:(c + 1) * BS, :R] = results[c]["out"].T
    return out


def _run(in_maps, **spmd_kwargs):
    from concourse.bass_utils import run_bass_kernel_spmd
    nc = _get_nc()
    return run_bass_kernel_spmd(nc, in_maps, core_ids=list(range(N_CORES)),
                                **spmd_kwargs)


def kernel(inputs, prev_state, W_in, W_res, W_gate):
    in_maps = _pack_inputs(inputs, prev_state, W_in, W_res, W_gate)
    res = _run(in_maps)
    return _assemble(res.results)
